# revision 18
# baseline (speedup 1.0000x reference)
"""Trainium2 Bass kernel for additive (Bahdanau-style) attention.

Reference computation (per batch element b):
    kx = keys[b] @ Wx.T                      # [L, M]
    qh = query @ Wh.T + bh                   # [L1, M]
    g  = relu(kx[None,:,:] + qh[:,None,:])   # [L1, L, M]
    s  = g @ w                               # [L1, L]
    e  = softmax(s, axis=-1)
    out[b] = e @ values[b]                   # [L1, D]

Sharding: batch (B=8) across the 8 NeuronCores, one batch element per core.
query/Wx/Wh/bh/w are replicated (tiny).

Per-core algorithm (v9).  Measured HW facts this is built around:
  * DVE tensor_scalar [128,1024] bf16 ~406 ns effective; ACT Relu ~1032 --
    the two engines' joint throughput (256 units ~75 us) is the roofline.
    GPSIMD elementwise is an 8-lane software path (~15 us/unit; unusable).
  * PE HAM clock gate is binary 1.2/2.4 GHz: flips warm after ~3.4 us of
    SUSTAINED busy, cools after ~3.4 us idle.  Junk matmuls ramp it during
    the input DMA window so the kx/qh/score matmuls all run at 2.4 GHz.
  * A single DMA moves ~90 GB/s (it stripes over ~4 of 16 SDMA engines),
    same-queue DMAs serialize, and sub-1KB descriptors cost ~40 ns each.
    Therefore every tensor is PRE-PERMUTED ON THE HOST into its exact SBUF
    layout (dense [128, K] copies, 1-8 KB lines), keys/Wx ship as fp8-e4m3
    (halving the critical bytes; Wx,Wh,bh are host-scaled x32 and w /32 --
    relu's positive homogeneity makes this exact), and the transfers are
    spread by need-time over the three issue queues (SP / ACT / GPSIMD):
    only kt + m-slice-0 of Wx/Wh + query gate the first g unit; m>=1
    weight slices follow; values is token-deferred into the main loop.
  * g units: relu(kxT_tile + qhT[:, q]) as [128,1024] per-partition-bias
    ops, DVE/ACT split 186/70, m-OUTER so only kx m-tile 0 gates the start;
    kx/qh matmuls for tile m+1 run inside block m on the warm PE, their
    PSUM->bf16 casts at the top of block m+1.
  * scores: PE matmuls reduce over m (partitions); stationary operand is a
    window of a zero-padded copy of w so query (16j + c)'s score row lands
    at PSUM partition 32j + c; four concurrent column-tiled matmuls
    (tile_position (0,32j)) stream four g tensors at once.
  * softmax without max-subtraction (scores are O(1)); Exp writes bf16 e;
    bf16 single-pass transposes (bf16 identity); row sums via DVE reduces;
    bf16 e.T @ bf16 values matmul; 1/sum row-scale + output DMA in halves.
  * The 64 unused PSUM rows carry garbage that never reaches the output:
    the host gathers the 64 valid rows (ROW_OF_Q) per core.
"""

import numpy as np

import concourse.bacc as bacc
import concourse.mybir as mybir
import concourse.tile as tile
from concourse.bass_utils import run_bass_kernel_spmd
from concourse.masks import make_identity

B, L1, L, D, M = 8, 64, 1024, 512, 512
N_CORES = 8

FP32 = mybir.dt.float32
BF16 = mybir.dt.bfloat16
F8E4 = mybir.dt.float8e4
AF = mybir.ActivationFunctionType
OP = mybir.AluOpType

NJ = 4  # column groups
NC = 16  # c values per column group (NJ * NC == L1)

N_JUNK_BIG = 8  # [128,512] cold-clock ramp matmuls (~3.6us -> HAM warm)
N_JUNK_SMALL = 2  # [128,128] warm-hold matmuls until kt lands

WSCALE = 32.0  # host scale on Wx/Wh/bh (and 1/WSCALE on w): keeps fp8 Wx
# out of the subnormal range; exact by relu positive homogeneity


def _engine_of(c, j, m):
    """Static engine split for the (c, j, m) g-unit slot: DVE 189 / ACT 67."""
    if j == 3:
        return "A"
    if j == 2 and c == 15 and m < 3:
        return "A"
    return "D"


def build_kernel():
    nc = bacc.Bacc()

    # All inputs arrive pre-permuted into SBUF layout (dense [128, K]).
    ktp = nc.declare_dram_parameter("ktp", [128, 4 * L], F8E4, isOutput=False)
    wxp = nc.declare_dram_parameter("wxp", [128, 4 * M], F8E4, isOutput=False)
    whp = nc.declare_dram_parameter("whp", [128, 4 * M], F8E4, isOutput=False)
    qtp = nc.declare_dram_parameter("qtp", [128, 4 * L1], F8E4, isOutput=False)
    vtp = nc.declare_dram_parameter("vtp", [128, 8 * D], BF16, isOutput=False)
    bh2 = nc.declare_dram_parameter("bh2", [128, 4], FP32, isOutput=False)
    w2 = nc.declare_dram_parameter("w2", [128, 4], FP32, isOutput=False)
    out = nc.declare_dram_parameter("out", [128, D], FP32, isOutput=True)

    with tile.TileContext(nc) as tc:
        with (
            tc.tile_pool(name="const", bufs=1) as cp,
            tc.tile_pool(name="g", bufs=8) as gp,
            tc.tile_pool(name="pk", bufs=2, space="PSUM") as pp_k,
            tc.tile_pool(name="pt", bufs=2, space="PSUM") as pp_t,
            tc.tile_pool(name="po", bufs=1, space="PSUM") as pp_o,
            tc.tile_pool(name="pq", bufs=1, space="PSUM") as pp_q,
            tc.tile_pool(name="psc", bufs=1, space="PSUM") as pp_s,
        ):
            # ---- persistent SBUF tensors
            # wx/wh are M-MAJOR: column m*512 + a*128 + i holds row a*128+p,
            # col m*128+i of the transposed weight -- so the m=0 slice is one
            # contiguous [128, 512] block (single big-line DMA).
            wx = cp.tile([128, 4 * M], F8E4, name="wx")
            padA = cp.tile([128, 4 * M], F8E4, name="padA")
            kt = cp.tile([128, 4 * L], F8E4, name="kt")
            padB = cp.tile([128, 4 * L], F8E4, name="padB")
            wh = cp.tile([128, 4 * M], F8E4, name="wh")
            padC = cp.tile([128, 4 * M], F8E4, name="padC")
            qt = cp.tile([128, 4 * L1], F8E4, name="qt")
            padD = cp.tile([128, 4 * L1], F8E4, name="padD")
            bhs = cp.tile([128, 4], FP32, name="bhs")
            w2s = cp.tile([128, 4], FP32, name="w2s")
            vt = cp.tile([128, 8 * D], BF16, name="vt")
            kxbf = cp.tile([128, 4 * L], BF16, name="kxbf")
            qhf = cp.tile([128, 4 * L1], FP32, name="qhf")
            w2bf = cp.tile([128, 4], BF16, name="w2bf")
            wpad = cp.tile([128, 4 * 65], BF16, name="wpad")
            identb = cp.tile([128, 128], BF16, name="identb")
            e_sb = cp.tile([128, L], BF16, name="e_sb")
            eT = cp.tile([128, L], BF16, name="eT")
            ssum2 = cp.tile([128, 2], FP32, name="ssum2")
            ssum = cp.tile([128, 1], FP32, name="ssum")
            rs = cp.tile([128, 1], FP32, name="rs")
            out_sb = cp.tile([128, D], FP32, name="out_sb")
            junk_a = cp.tile([128, 128], BF16, name="junk_a")
            junk_b = cp.tile([128, 512], BF16, name="junk_b")

            # ---- input DMAs by need-time across the three issue queues.
            # critical for the first g unit: kt (both halves), wx m-slice 0,
            # wh m-slice 0, qt.  m>=1 slices needed ~17+ us in; values ~90 us.
            nc.sync.dma_start(kt[:, 0:1024], ktp[:, 0:1024])
            nc.gpsimd.dma_start(kt[:, 1024:2048], ktp[:, 1024:2048])
            nc.sync.dma_start(kt[:, 2048:3072], ktp[:, 2048:3072])
            nc.gpsimd.dma_start(kt[:, 3072:4096], ktp[:, 3072:4096])
            nc.scalar.dma_start(wx[:, 0:512], wxp[:, 0:512])
            nc.scalar.dma_start(wh[:, 0:512], whp[:, 0:512])
            nc.scalar.dma_start(qt[:], qtp[:, :])
            nc.scalar.dma_start(bhs[:], bh2[:, :])
            nc.scalar.dma_start(w2s[:], w2[:, :])
            nc.gpsimd.dma_start(wx[:, 512:2048], wxp[:, 512:2048])
            nc.sync.dma_start(wh[:, 512:2048], whp[:, 512:2048])
            # (kt is lc-major: columns lc*2048 + a*512 + i, so each half is
            # one contiguous 2 KB-per-partition transfer)

            # ---- junk operand memsets on DVE (no deps -> immediate), so the
            # PE warmup starts right after the preamble
            nc.vector.memset(junk_a[:], 0.0)
            nc.vector.memset(junk_b[:], 0.0)
            make_identity(nc, identb[:])

            # ---- PE warm-up: ramp the HAM clock gate with big junk matmuls,
            # then hold it warm with small ones until the input DMAs land.
            pwarm = pp_s.tile([128, L], FP32, tag="ps", name="warm")
            for r in range(N_JUNK_BIG):
                nc.tensor.matmul(
                    pwarm[:, 0:512], junk_a[:], junk_b[:], start=True, stop=True
                )
            for r in range(N_JUNK_SMALL):
                nc.tensor.matmul(
                    pwarm[:, 0:128],
                    junk_a[:],
                    junk_b[:, 0:128],
                    start=True,
                    stop=True,
                )

            # ---- small prep (vector engine)
            nc.vector.tensor_copy(w2bf[:], w2s[:])
            nc.vector.memset(wpad[:], 0.0)
            for m in range(4):
                nc.vector.tensor_copy(
                    wpad[:, 65 * m + 32 : 65 * m + 33], w2bf[:, m : m + 1]
                )

            # ---- PE kx matmuls for one m-tile (PSUM); casts separate
            kx_psum = {}

            def kx_mm_lc(m, lc):
                pk = pp_k.tile([128, 512], FP32, tag="pk", name=f"pk{m}{lc}")
                for a in range(4):
                    nc.tensor.matmul(
                        pk[:],
                        wx[:, 512 * m + 128 * a : 512 * m + 128 * (a + 1)],
                        kt[:, 2048 * lc + 512 * a : 2048 * lc + 512 * (a + 1)],
                        start=(a == 0),
                        stop=(a == 3),
                    )
                kx_psum[(m, lc)] = pk

            def kx_mm(m):
                kx_mm_lc(m, 0)
                kx_mm_lc(m, 1)

            def kx_cast(m):
                # lc0 on DVE, lc1 on ACT
                for lc in range(2):
                    dst = kxbf[:, L * m + 512 * lc : L * m + 512 * (lc + 1)]
                    if lc == 1:
                        nc.scalar.copy(dst, kx_psum[(m, lc)][:])
                    else:
                        nc.vector.tensor_copy(dst, kx_psum[(m, lc)][:])

            # qh matmuls for one m-tile + bias-add on ACT
            pq = pp_q.tile([128, 4 * L1], FP32, tag="pq", name="pq")

            def qh_mm(m):
                for a in range(4):
                    nc.tensor.matmul(
                        pq[:, L1 * m : L1 * (m + 1)],
                        wh[:, 512 * m + 128 * a : 512 * m + 128 * (a + 1)],
                        qt[:, L1 * a : L1 * (a + 1)],
                        start=(a == 0),
                        stop=(a == 3),
                    )
                nc.scalar.activation(
                    qhf[:, L1 * m : L1 * (m + 1)],
                    pq[:, L1 * m : L1 * (m + 1)],
                    AF.Identity,
                    bias=bhs[:, m : m + 1],
                )

            kx_mm_lc(0, 0)
            qh_mm(0)
            kx_mm_lc(0, 1)
            nc.vector.tensor_copy(kxbf[:, 0:512], kx_psum[(0, 0)][:])

            # ---- main stage: g units (DVE+ACT) + score matmuls, m-OUTER.
            # query q = 16j + c accumulates its scores into PSUM row 32j + c.
            ps = pp_s.tile([128, L], FP32, tag="ps", name="ps")
            _half_tiles = []
            for m in range(4):
                if m > 0:
                    kx_cast(m)
                for c in range(NC):
                    if c == 6 and m < 3:
                        kx_mm(m + 1)
                    if c == 10 and m < 3:
                        qh_mm(m + 1)
                    if c == 8 and m == 0:
                        # token read of qhf m0 (already written) creates the
                        # WAR dep that delays the 2 MB values DMA until the
                        # head transfers have drained
                        nc.vector.tensor_copy(vt[:, 0:1], qhf[:, 63:64])
                        nc.vector.tensor_copy(vt[:, 2048:2049], qhf[:, 63:64])
                        nc.sync.dma_start(vt[:, 0:2048], vtp[:, 0:2048])
                        nc.gpsimd.dma_start(vt[:, 2048:4096], vtp[:, 2048:4096])
                    g4 = gp.tile([128, NJ * L], BF16, tag="g", name=f"g{c}_{m}")
                    kx_sl = kxbf[:, L * m : L * (m + 1)]

                    def emit_unit(j, lo, hi, c=c, m=m, g4=g4):
                        q = NC * j + c
                        gt = g4[:, L * j + lo : L * j + hi]
                        src_ = kxbf[:, L * m + lo : L * m + hi]
                        if _engine_of(c, j, m) == "A":
                            nc.scalar.activation(
                                gt,
                                src_,
                                AF.Relu,
                                bias=qhf[:, L1 * m + q : L1 * m + q + 1],
                            )
                        else:
                            nc.vector.tensor_scalar(
                                gt,
                                src_,
                                qhf[:, L1 * m + q : L1 * m + q + 1],
                                0.0,
                                op0=OP.add,
                                op1=OP.max,
                            )

                    if m == 0 and c < 3:
                        # warm-up c-groups: lc0 halves first (kt lc1 is still
                        # in flight / its cast pending), then lc1 halves.
                        # cast(0,1) on DVE lands between c1 and c2 half-runs.
                        _half_tiles.append(g4)
                        for j in range(NJ):
                            emit_unit(j, 0, 512)
                        if c == 1:
                            nc.vector.tensor_copy(
                                kxbf[:, 512:1024], kx_psum[(0, 1)][:]
                            )
                        if c == 2:
                            for cc in range(3):
                                g4cc = _half_tiles[cc]
                                for j in range(NJ):
                                    q = NC * j + cc
                                    gt = g4cc[:, L * j + 512 : L * j + 1024]
                                    if _engine_of(cc, j, 0) == "A":
                                        nc.scalar.activation(
                                            gt,
                                            kxbf[:, 512:1024],
                                            AF.Relu,
                                            bias=qhf[:, q : q + 1],
                                        )
                                    else:
                                        nc.vector.tensor_scalar(
                                            gt,
                                            kxbf[:, 512:1024],
                                            qhf[:, q : q + 1],
                                            0.0,
                                            op0=OP.add,
                                            op1=OP.max,
                                        )
                    else:
                        for j in range(NJ):
                            emit_unit(j, 0, L)
                    def score_mms(cc, lc, g4cc, m=m):
                        for j in range(NJ):
                            nc.tensor.matmul(
                                ps[32 * j : 32 * (j + 1), 512 * lc : 512 * (lc + 1)],
                                wpad[:, 65 * m + 32 - cc : 65 * m + 64 - cc],
                                g4cc[:, L * j + 512 * lc : L * j + 512 * (lc + 1)],
                                start=(cc == 0 and m == 0),
                                stop=(cc == NC - 1 and m == 3),
                                tile_position=(0, 32 * j),
                            )

                    if m == 0 and c < 3:
                        score_mms(c, 0, g4)
                        if c == 2:
                            for cc in range(3):
                                score_mms(cc, 1, _half_tiles[cc])
                    else:
                        score_mms(c, 0, g4)
                        score_mms(c, 1, g4)

            # ---- softmax (no max subtraction; scores are O(1)) + epilogue,
            # pipelined per 128-column chunk: exp -> transpose -> copy -> mm
            po = pp_o.tile([128, D], FP32, name="po")
            for a in range(8):
                if a % 4 == 0:
                    nc.scalar.activation(
                        e_sb[:, 128 * a : 128 * (a + 4)],
                        ps[:, 128 * a : 128 * (a + 4)],
                        AF.Exp,
                    )
                pt = pp_t.tile([128, 128], BF16, tag="pt", name=f"pt{a}")
                nc.tensor.transpose(pt[:], e_sb[:, 128 * a : 128 * (a + 1)], identb[:])
                nc.vector.tensor_copy(eT[:, 128 * a : 128 * (a + 1)], pt[:])
                nc.tensor.matmul(
                    po[:],
                    eT[:, 128 * a : 128 * (a + 1)],
                    vt[:, D * a : D * (a + 1)],
                    start=(a == 0),
                    stop=(a == 7),
                )
            nc.vector.reduce_sum(
                ssum2[:, 0:1], e_sb[:, 0:512], axis=mybir.AxisListType.X
            )
            nc.vector.reduce_sum(
                ssum2[:, 1:2], e_sb[:, 512:1024], axis=mybir.AxisListType.X
            )
            nc.vector.reduce_sum(ssum[:], ssum2[:], axis=mybir.AxisListType.X)
            nc.vector.reciprocal(rs[:], ssum[:])
            for h in range(4):
                osl = out_sb[:, 128 * h : 128 * (h + 1)]
                psl = po[:, 128 * h : 128 * (h + 1)]
                if h % 2 == 0:
                    nc.vector.tensor_scalar(
                        osl, psl, rs[:], None, op0=OP.mult
                    )
                else:
                    nc.scalar.activation(psl_out := osl, psl, AF.Copy, scale=rs[:])
                if h % 2 == 1:
                    nc.sync.dma_start(
                        out[:, 128 * (h - 1) : 128 * (h + 1)],
                        out_sb[:, 128 * (h - 1) : 128 * (h + 1)],
                    )

    nc.finalize()
    return nc


_NC_CACHE = {}


def get_nc():
    if "nc" not in _NC_CACHE:
        _NC_CACHE["nc"] = build_kernel()
    return _NC_CACHE["nc"]


def _perm_weight(WT, dtype):
    """[D, M] transposed weight -> m-major SBUF image [128, 4*M]:
    out[p, m*512 + a*128 + i] = WT[a*128 + p, m*128 + i]."""
    D_, M_ = WT.shape
    t = WT.reshape(4, 128, 4, 128)  # [a, p, m, i]
    t = np.transpose(t, (1, 2, 0, 3))  # [p, m, a, i]
    return np.ascontiguousarray(t.reshape(128, 4 * M_).astype(dtype))


def _perm_amajor(XT, dtype):
    """[R, C] with R = 4*128 -> a-major SBUF image [128, 4*C]:
    out[p, a*C + c] = XT[a*128 + p, c]."""
    R, C = XT.shape
    t = XT.reshape(4, 128, C)  # [a, p, c]
    t = np.transpose(t, (1, 0, 2))  # [p, a, c]
    return np.ascontiguousarray(t.reshape(128, 4 * C).astype(dtype))


def make_in_maps(query, keys, values, Wx, Wh, bh, w):
    import ml_dtypes

    bf16 = ml_dtypes.bfloat16
    f8 = ml_dtypes.float8_e4m3fn
    query = np.asarray(query, dtype=np.float32)
    keys = np.asarray(keys, dtype=np.float32)
    values = np.asarray(values, dtype=np.float32)
    Wx = np.asarray(Wx, dtype=np.float32)
    Wh = np.asarray(Wh, dtype=np.float32)
    bh = np.asarray(bh, dtype=np.float32)
    w = np.asarray(w, dtype=np.float32)

    wxp = _perm_weight(Wx.T * WSCALE, f8)
    whp = _perm_weight(Wh.T * WSCALE, f8)
    qtp = _perm_amajor(query.T, f8)
    bh2 = np.ascontiguousarray((bh * WSCALE).reshape(4, 128).T)
    w2 = np.ascontiguousarray((w / WSCALE).reshape(4, 128).T)

    in_maps = []
    for c in range(N_CORES):
        kc = keys[c].reshape(2, 512, 4, 128)  # [lc, i, a, p]
        ktp = np.ascontiguousarray(
            np.transpose(kc, (3, 0, 2, 1)).reshape(128, 4 * L).astype(f8)
        )
        v8 = values[c].reshape(8, 128, D)  # [a, p, d]
        vtp = np.ascontiguousarray(
            np.transpose(v8, (1, 0, 2)).reshape(128, 8 * D).astype(bf16)
        )
        in_maps.append(
            {
                "ktp": ktp,
                "vtp": vtp,
                "qtp": qtp,
                "wxp": wxp,
                "whp": whp,
                "bh2": bh2,
                "w2": w2,
            }
        )
    return in_maps


def run(in_maps, **kwargs):
    nc = get_nc()
    return run_bass_kernel_spmd(nc, in_maps, core_ids=list(range(N_CORES)), **kwargs)


ROW_OF_Q = np.array([32 * (q // NC) + q % NC for q in range(L1)])


def kernel(query, keys, values, Wx, Wh, bh, w):
    in_maps = make_in_maps(query, keys, values, Wx, Wh, bh, w)
    res = run(in_maps)
    return np.stack(
        [res.results[c]["out"][ROW_OF_Q, :] for c in range(N_CORES)], axis=0
    )
